# revision 1
# baseline (speedup 1.0000x reference)
"""GQA attention with sliding-window+sink KV slicing on 8 trn2 NeuronCores.

Sharding: core = (batch b in 0..1, query-chunk c in 0..3); each core handles
1024 query tokens of one batch against the full sliced KV (sink 4 + window
1024 = 1028 positions), with all weights replicated (streamed from HBM).
No cross-core collectives; host concatenates the 8 output chunks.

Per-core pipeline (all matmuls bf16 with fp32 PSUM accumulation):
  1. q = x_chunk @ wq.T   -> [t,qd] tiles, RoPE applied along free dim,
     then PE-transposed to q_T [qd, t] (contraction layout for QK^T).
  2. k = x_kv @ wk.T      -> RoPE'd, transposed to k_T [kd, s], duplicated
     into both partition halves so head pairs can row-pack the PE array.
  3. v = x_kv @ wv.T      -> [s, vd] with a ones column per head (gives the
     softmax denominator for free during the PV matmul).
  4. scores_T[s,t] = k_T/q_T matmul (2 heads packed via tile_position);
     probs = Exp(scores/8) on ScalarE (no max subtraction needed: |scores|
     is bounded ~6 for this problem's weight scale); multiplicative mask
     implements torch-SDPA top-left-aligned causal over the sliced KV.
  5. ctx_aug[65,t] = v_aug.T @ probs accumulated over s-tiles; row 64 is the
     denominator; reciprocal is broadcast back over head rows with a tiny
     selector matmul, applied on VectorE.
  6. out = ctx_norm-major matmul with wproj, written fp32.
"""

import numpy as np
import ml_dtypes

import concourse.bass as bass
import concourse.bacc as bacc
import concourse.tile as tile
import concourse.mybir as mybir
from concourse.bass_utils import run_bass_kernel_spmd

BF = mybir.dt.bfloat16
F32 = mybir.dt.float32
BF_NP = ml_dtypes.bfloat16

# problem constants
D_MODEL = 2048
N_HEADS = 32
N_KV = 8
D_HEAD = 64
GROUP = 4
B, T = 2, 4096
WINDOW = 1024
SINK = 4
ROPE_BASE = 10000.0

# sharding/tiling constants
NCORES = 8
TQ = 1024             # query tokens per core
S = SINK + WINDOW     # 1028 kv positions
DMT = D_MODEL // 128  # 16 contraction tiles
TT = TQ // 128        # 8 query 128-subtiles
TC = TQ // 512        # 2 query 512-chunks
QC = D_MODEL // 512   # 4 qd 512-chunks
ST = (S + 127) // 128  # 9 s-tiles (last has 4 rows)
HP = N_HEADS // 2     # 16 head pairs
KB = N_KV // 2        # 4 kv pair tiles
SCALE = float(1.0 / np.sqrt(D_HEAD))

_CACHED = {}


def _sp(j):
    return 128 if j < ST - 1 else S - 128 * (ST - 1)


def _ins0(ap, dim_idx, n):
    """Return a copy of `ap` with a step-0 (broadcast) dim inserted at
    free-dim position `dim_idx` (0 = right after the partition dim)."""
    dims = list(ap.ap)
    dims.insert(1 + dim_idx, [0, n])
    return bass.AP(tensor=ap.tensor, offset=ap.offset, ap=dims)


def _set0(ap, dim_idx, n):
    """Replace free dim `dim_idx` (which must have count 1) by [0, n]."""
    dims = list(ap.ap)
    assert dims[1 + dim_idx][1] == 1
    dims[1 + dim_idx] = [0, n]
    return bass.AP(tensor=ap.tensor, offset=ap.offset, ap=dims)


def _build_bass():
    nc = bacc.Bacc("TRN2", target_bir_lowering=False, debug=False,
                   num_devices=NCORES)

    def din(name, shape, dt=BF):
        return nc.dram_tensor(name, shape, dt, kind="ExternalInput").ap()

    XQ = din("xq_t", [128, DMT * TQ])
    XKV = din("xkv_t", [128, DMT * S])
    WQ = din("wq_t", [128, QC * DMT * 512])
    WK = din("wk_t", [128, DMT * 512])
    WV = din("wv_t", [128, DMT * 512])
    WP = din("wp_t", [128, QC * DMT * 512])
    COSQ = din("cosq_t", [128, TT * 32], F32)
    SINQ = din("sinq_t", [128, TT * 32], F32)
    COSK = din("cosk_t", [128, ST * 32], F32)
    SINK_T = din("sink_t", [128, ST * 32], F32)
    MB = din("maskb_t", [128, ST * TC * 512])
    SEL = din("sel_t", [32, HP * 128])
    IDENT = din("ident_t", [128, 128])
    OUT = nc.dram_tensor("out", [TQ, D_MODEL], F32, kind="ExternalOutput").ap()

    with tile.TileContext(nc) as tc:
        _body(tc, XQ, XKV, WQ, WK, WV, WP, COSQ, SINQ, COSK, SINK_T, MB, SEL,
              IDENT, OUT)
    nc.compile()
    return nc


def _rope(nc, rtp, src_ap, dst_ap, cos_ap, sin_ap, sp):
    """src [sp, 8*64] (PSUM f32) -> dst [sp, 8*64] (SBUF bf16), rotate-half
    RoPE along the free dim.  cos_ap/sin_ap: [sp, 32] broadcast over heads."""
    s3 = src_ap.rearrange("p (h d) -> p h d", h=8)
    d3 = dst_ap.rearrange("p (h d) -> p h d", h=8)
    cos = _ins0(cos_ap, 0, 8)
    sin = _ins0(sin_ap, 0, 8)
    h1, h2 = s3[:, :, 0:32], s3[:, :, 32:64]
    t1 = rtp.tile([128, 8 * 32], F32, tag="rt1")
    t2 = rtp.tile([128, 8 * 32], F32, tag="rt2")
    t13 = t1[:sp, :].rearrange("p (h d) -> p h d", h=8)
    t23 = t2[:sp, :].rearrange("p (h d) -> p h d", h=8)
    nc.vector.tensor_mul(t13, h2, sin)
    nc.vector.tensor_mul(t23, h1, cos)
    nc.vector.tensor_sub(d3[:, :, 0:32], t23, t13)
    t1b = rtp.tile([128, 8 * 32], F32, tag="rt1")
    t2b = rtp.tile([128, 8 * 32], F32, tag="rt2")
    t13b = t1b[:sp, :].rearrange("p (h d) -> p h d", h=8)
    t23b = t2b[:sp, :].rearrange("p (h d) -> p h d", h=8)
    nc.vector.tensor_mul(t13b, h1, sin)
    nc.vector.tensor_mul(t23b, h2, cos)
    nc.vector.tensor_add(d3[:, :, 32:64], t23b, t13b)


def _body(tc, XQ, XKV, WQ, WK, WV, WP, COSQ, SINQ, COSK, SINK_T, MB, SEL,
          IDENT, OUT):
    nc = tc.nc

    def load(pool, name, src, shape, dt=BF):
        t = pool.tile(shape, dt, tag=name)
        nc.sync.dma_start(out=t[:, :], in_=src)
        return t

    with tc.tile_pool(name="life", bufs=1) as life:
        qT_sb = life.tile([128, HP * TQ], BF, tag="qT")    # [qd-pair, t]
        kT_sb = life.tile([128, N_KV * S], BF, tag="kT")   # [kd dup-halves, s]
        v_sb = life.tile([128, ST * N_KV * 65], BF, tag="v")
        ctx_sb = life.tile([128, HP * TQ], BF, tag="ctx")  # [dm, t]
        dens_sb = life.tile([32, TQ], BF, tag="dens")  # row 2hp=den(B), 2hp+1=den(A)
        ident_sb = load(life, "ident_sb", IDENT, [128, 128])

        # ================= phase 1: Q proj + rope + transpose =============
        with (
            tc.tile_pool(name="p1s", bufs=1) as p1s,
            tc.tile_pool(name="wst1", bufs=2) as wst1,
            tc.tile_pool(name="rtp1", bufs=4) as rtp1,
            tc.tile_pool(name="qrp1", bufs=3) as qrp1,
            tc.tile_pool(name="ppq1", bufs=2, space="PSUM") as ppq1,
            tc.tile_pool(name="ptr1", bufs=2, space="PSUM") as ptr1,
        ):
            xq_sb = load(p1s, "xq_sb", XQ, [128, DMT * TQ])
            cosq_sb = load(p1s, "cosq_sb", COSQ, [128, TT * 32], F32)
            sinq_sb = load(p1s, "sinq_sb", SINQ, [128, TT * 32], F32)
            for qc in range(QC):
                wblk = wst1.tile([128, DMT * 512], BF, tag="wblk")
                nc.sync.dma_start(
                    out=wblk[:, :],
                    in_=WQ[:, qc * DMT * 512:(qc + 1) * DMT * 512])
                for tt in range(TT):
                    pq = ppq1.tile([128, 512], F32, tag="pproj")
                    for dm in range(DMT):
                        nc.tensor.matmul(
                            pq[:, :],
                            lhsT=xq_sb[:, dm * TQ + tt * 128:
                                       dm * TQ + tt * 128 + 128],
                            rhs=wblk[:, dm * 512:(dm + 1) * 512],
                            start=(dm == 0), stop=(dm == DMT - 1))
                    qr = qrp1.tile([128, 512], BF, tag="qrope")
                    _rope(nc, rtp1, pq[:, :], qr[:, :],
                          cosq_sb[:, tt * 32:(tt + 1) * 32],
                          sinq_sb[:, tt * 32:(tt + 1) * 32], 128)
                    for qi in range(4):
                        qb = qc * 4 + qi  # head-pair index
                        pt = ptr1.tile([128, 128], BF, tag="ptrn")
                        nc.tensor.transpose(pt[:, :],
                                            qr[:, qi * 128:(qi + 1) * 128],
                                            ident_sb[:, :])
                        nc.vector.tensor_copy(
                            qT_sb[:, qb * TQ + tt * 128:
                                  qb * TQ + tt * 128 + 128],
                            pt[:, :])

        # ================= phase 2+3: K/V proj ============================
        with (
            tc.tile_pool(name="p2s", bufs=1) as p2s,
            tc.tile_pool(name="rtp2", bufs=4) as rtp2,
            tc.tile_pool(name="qrp2", bufs=3) as qrp2,
            tc.tile_pool(name="ppq2", bufs=2, space="PSUM") as ppq2,
            tc.tile_pool(name="ptr2", bufs=2, space="PSUM") as ptr2,
        ):
            xkv_sb = load(p2s, "xkv_sb", XKV, [128, DMT * S])
            wk_sb = load(p2s, "wk_sb", WK, [128, DMT * 512])
            wv_sb = load(p2s, "wv_sb", WV, [128, DMT * 512])
            cosk_sb = load(p2s, "cosk_sb", COSK, [128, ST * 32], F32)
            sink_sb = load(p2s, "sink_sb", SINK_T, [128, ST * 32], F32)

            for ss in range(ST):
                sp = _sp(ss)
                pk = ppq2.tile([128, 512], F32, tag="pproj")
                for dm in range(DMT):
                    nc.tensor.matmul(
                        pk[:sp, :],
                        lhsT=xkv_sb[:, dm * S + ss * 128:
                                    dm * S + ss * 128 + sp],
                        rhs=wk_sb[:, dm * 512:(dm + 1) * 512],
                        start=(dm == 0), stop=(dm == DMT - 1))
                kr = qrp2.tile([128, 512], BF, tag="qrope")
                _rope(nc, rtp2, pk[:sp, :], kr[:sp, :],
                      cosk_sb[:sp, ss * 32:(ss + 1) * 32],
                      sink_sb[:sp, ss * 32:(ss + 1) * 32], sp)
                for kb in range(KB):
                    pt = ptr2.tile([128, 128], BF, tag="ptrn")
                    nc.tensor.transpose(pt[:, :sp],
                                        kr[:sp, kb * 128:(kb + 1) * 128],
                                        ident_sb[:sp, :sp])
                    gA, gB = 2 * kb, 2 * kb + 1
                    nc.vector.tensor_copy(
                        kT_sb[0:64, gA * S + ss * 128: gA * S + ss * 128 + sp],
                        pt[0:64, :sp])
                    nc.vector.tensor_copy(
                        kT_sb[64:128, gB * S + ss * 128:
                              gB * S + ss * 128 + sp],
                        pt[64:128, :sp])
            # duplicate each kv head's k rows into the other partition half
            for g in range(N_KV):
                if g % 2 == 0:
                    nc.sync.dma_start(out=kT_sb[64:128, g * S:(g + 1) * S],
                                      in_=kT_sb[0:64, g * S:(g + 1) * S])
                else:
                    nc.sync.dma_start(out=kT_sb[0:64, g * S:(g + 1) * S],
                                      in_=kT_sb[64:128, g * S:(g + 1) * S])

            # V projection (+ ones columns)
            v4 = v_sb[:, :].rearrange("p (s h c) -> p s h c", s=ST, c=65)
            for ss in range(ST):
                sp = _sp(ss)
                pv = ppq2.tile([128, 512], F32, tag="pproj")
                for dm in range(DMT):
                    nc.tensor.matmul(
                        pv[:sp, :],
                        lhsT=xkv_sb[:, dm * S + ss * 128:
                                    dm * S + ss * 128 + sp],
                        rhs=wv_sb[:, dm * 512:(dm + 1) * 512],
                        start=(dm == 0), stop=(dm == DMT - 1))
                nc.vector.tensor_copy(
                    v4[:sp, ss, :, 0:64],
                    pv[:sp, :].rearrange("p (h d) -> p h d", h=8))
                nc.vector.memset(v4[:sp, ss, :, 64:65], 1.0)

        # ================= phase 4: attention =============================
        with (
            tc.tile_pool(name="p4s", bufs=1) as p4s,
            tc.tile_pool(name="prb", bufs=2) as prb,
            tc.tile_pool(name="stg", bufs=3) as stg,
            tc.tile_pool(name="psc", bufs=2, space="PSUM") as psc,
            tc.tile_pool(name="ppv", bufs=2, space="PSUM") as ppv,
        ):
            mb_sb = load(p4s, "mb_sb", MB, [128, ST * TC * 512])
            mb4 = mb_sb[:, :].rearrange("p (s t v) -> p s t v", s=ST, t=TC)
            for hp in range(HP):
                g = hp // 2  # shared kv head for both heads of the pair
                for tcq in range(TC):
                    pvt = ppv.tile([128, 1024], F32, tag="pvacc")
                    probs = prb.tile([128, ST * 1024], BF, tag="probs")
                    p4 = probs[:, :].rearrange("p (s u v) -> p s u v",
                                               s=ST, u=2)
                    for j in range(ST):
                        sp = _sp(j)
                        sc = psc.tile([128, 1024], F32, tag="scores")
                        nc.tensor.matmul(
                            sc[:sp, 0:512],
                            lhsT=kT_sb[0:64, g * S + j * 128:
                                       g * S + j * 128 + sp],
                            rhs=qT_sb[0:64, hp * TQ + tcq * 512:
                                      hp * TQ + tcq * 512 + 512],
                            start=True, stop=True, tile_position=(0, 0))
                        nc.tensor.matmul(
                            sc[:sp, 512:1024],
                            lhsT=kT_sb[64:128, g * S + j * 128:
                                       g * S + j * 128 + sp],
                            rhs=qT_sb[64:128, hp * TQ + tcq * 512:
                                      hp * TQ + tcq * 512 + 512],
                            start=True, stop=True, tile_position=(64, 0))
                        nc.scalar.activation(
                            p4[:sp, j, :, :],
                            sc[:sp, :].rearrange("p (u v) -> p u v", u=2),
                            mybir.ActivationFunctionType.Exp, scale=SCALE)
                    # multiplicative causal mask (shared across head pair)
                    mfull = _ins0(mb4[:, 0:ST - 1, tcq, :], 1, 2)
                    nc.vector.tensor_mul(p4[:, 0:ST - 1, :, :],
                                         p4[:, 0:ST - 1, :, :], mfull)
                    spl = _sp(ST - 1)
                    mlast = _ins0(mb4[:spl, ST - 1, tcq, :], 0, 2)
                    nc.vector.tensor_mul(p4[:spl, ST - 1, :, :],
                                         p4[:spl, ST - 1, :, :], mlast)
                    # PV: ctx_aug[65, t] per head; denominator in row 64
                    for j in range(ST):
                        sp = _sp(j)
                        nc.tensor.matmul(
                            pvt[0:65, 0:512],
                            lhsT=v4[:sp, j, g, :],
                            rhs=p4[:sp, j, 0, :],
                            start=(j == 0), stop=(j == ST - 1))
                        nc.tensor.matmul(
                            pvt[0:65, 512:1024],
                            lhsT=v4[:sp, j, g, :],
                            rhs=p4[:sp, j, 1, :],
                            start=(j == 0), stop=(j == ST - 1))
                    # ctx head A plain copy (same partitions)
                    nc.vector.tensor_copy(
                        ctx_sb[0:64, hp * TQ + tcq * 512:
                               hp * TQ + tcq * 512 + 512],
                        pvt[0:64, 0:512])
                    # head B ctx (+den B in row 64) staged, den A beside it;
                    # DMA shifts B's rows to the upper partition half and
                    # scatters (denB, denA) into dens rows (2hp, 2hp+1).
                    stb = stg.tile([128, 1024], BF, tag="stageb")
                    nc.vector.tensor_copy(stb[0:65, 0:512],
                                          pvt[0:65, 512:1024])
                    nc.vector.tensor_copy(stb[64:65, 512:1024],
                                          pvt[64:65, 0:512])
                    nc.sync.dma_start(
                        out=ctx_sb[64:128, hp * TQ + tcq * 512:
                                   hp * TQ + tcq * 512 + 512],
                        in_=stb[0:64, 0:512])
                    nc.sync.dma_start(
                        out=dens_sb[2 * hp:2 * hp + 2,
                                    tcq * 512:(tcq + 1) * 512],
                        in_=stb[64:65, 0:1024])

        # ================= phase 5+6: normalize + out proj ================
        with (
            tc.tile_pool(name="p5s", bufs=1) as p5s,
            tc.tile_pool(name="wst2", bufs=2) as wst2,
            tc.tile_pool(name="osb", bufs=3) as osb,
            tc.tile_pool(name="pp5", bufs=2, space="PSUM") as pp5,
        ):
            sel_sb = load(p5s, "sel_sb", SEL, [32, HP * 128])
            densf_sb = p5s.tile([32, TQ], F32, tag="densf")
            rden_sb = p5s.tile([32, TQ], F32, tag="rden")
            rdenb_sb = p5s.tile([32, TQ], BF, tag="rdenb")
            nc.vector.tensor_copy(densf_sb[:, :], dens_sb[:, :])
            nc.vector.reciprocal(rden_sb[:, :], densf_sb[:, :])
            nc.vector.tensor_copy(rdenb_sb[:, :], rden_sb[:, :])
            for hp in range(HP):
                for tcq in range(TC):
                    pr = pp5.tile([128, 512], F32, tag="pproj")
                    nc.tensor.matmul(
                        pr[:, :],
                        lhsT=sel_sb[:, hp * 128:(hp + 1) * 128],
                        rhs=rdenb_sb[:, tcq * 512:(tcq + 1) * 512],
                        start=True, stop=True)
                    csl = ctx_sb[:, hp * TQ + tcq * 512:
                                 hp * TQ + tcq * 512 + 512]
                    nc.vector.tensor_mul(csl, csl, pr[:, :])
            for ob in range(QC):
                wblk = wst2.tile([128, DMT * 512], BF, tag="wblk2")
                nc.sync.dma_start(
                    out=wblk[:, :],
                    in_=WP[:, ob * DMT * 512:(ob + 1) * DMT * 512])
                for tt in range(TT):
                    po = pp5.tile([128, 512], F32, tag="pproj")
                    for hp in range(HP):
                        nc.tensor.matmul(
                            po[:, :],
                            lhsT=ctx_sb[:, hp * TQ + tt * 128:
                                        hp * TQ + tt * 128 + 128],
                            rhs=wblk[:, hp * 512:(hp + 1) * 512],
                            start=(hp == 0), stop=(hp == HP - 1))
                    ot = osb.tile([128, 512], F32, tag="outsb")
                    nc.vector.tensor_copy(ot[:, :], po[:, :])
                    nc.sync.dma_start(
                        out=OUT[tt * 128:(tt + 1) * 128,
                                ob * 512:(ob + 1) * 512],
                        in_=ot[:, :])


# ---------------------------------------------------------------------------
# host-side data prep
# ---------------------------------------------------------------------------

def _tile_weight_q(w):
    # w [2048, 2048] -> [128, QC*DMT*512]: [p][qc,dmt,o] = w[qc*512+o, dmt*128+p]
    return np.ascontiguousarray(
        w.reshape(QC, 512, DMT, 128).transpose(3, 0, 2, 1).reshape(128, -1)
    ).astype(BF_NP)


def _tile_weight_kv(w):
    # w [512, 2048] -> [128, DMT*512]: [p][dmt,o] = w[o, dmt*128+p]
    return np.ascontiguousarray(
        w.reshape(512, DMT, 128).transpose(2, 1, 0).reshape(128, -1)
    ).astype(BF_NP)


def _tile_x(xt):
    # xt [ntok, 2048] -> [128, DMT*ntok]: [p][dmt,t] = xt[t, dmt*128+p]
    n = xt.shape[0]
    return np.ascontiguousarray(
        xt.T.reshape(DMT, 128, n).transpose(1, 0, 2).reshape(128, -1)
    ).astype(BF_NP)


def _rope_tables(pos, ntile):
    # pos [ntile*128] -> cos,sin [128, ntile*32] f32
    invf = 1.0 / (ROPE_BASE ** (np.arange(0, D_HEAD, 2, dtype=np.float64) / D_HEAD))
    ang = pos.reshape(ntile, 128)[:, :, None] * invf[None, None, :]
    cos = np.ascontiguousarray(
        np.cos(ang).transpose(1, 0, 2).reshape(128, -1)).astype(np.float32)
    sin = np.ascontiguousarray(
        np.sin(ang).transpose(1, 0, 2).reshape(128, -1)).astype(np.float32)
    return cos, sin


def _core_inputs(x, shared, b, c):
    qoff = c * TQ
    xq = x[b, qoff:qoff + TQ]
    xkv = np.concatenate([x[b, :SINK], x[b, T - WINDOW:]], 0)

    qpos = (qoff + np.arange(TQ)).astype(np.float64)
    kpos = np.concatenate([np.arange(SINK), np.arange(T - WINDOW, T)]).astype(np.float64)
    kpos = np.concatenate([kpos, np.zeros(ST * 128 - S)])
    cosq, sinq = _rope_tables(qpos, TT)
    cosk, sink = _rope_tables(kpos, ST)

    # multiplicative mask [128, (j 9, tc 2, t 512)]
    p = np.arange(128)[:, None, None, None]
    jj = np.arange(ST)[None, :, None, None]
    tcq = np.arange(TC)[None, None, :, None]
    t = np.arange(512)[None, None, None, :]
    keep = ((tcq * 512 + t) >= (128 * jj + p - qoff)) & ((128 * jj + p) < S)
    mask = keep.astype(BF_NP)

    d = {
        "xq_t": _tile_x(xq),
        "xkv_t": _tile_x(xkv),
        "cosq_t": cosq, "sinq_t": sinq, "cosk_t": cosk, "sink_t": sink,
        "maskb_t": np.ascontiguousarray(mask.reshape(128, -1)),
    }
    d.update(shared)
    return d


def _prep_all(x, wq, wk, wv, w_proj):
    # dens rows are swizzled: row 2hp holds den(head B), 2hp+1 den(head A)
    sel = np.zeros((32, HP * 128), dtype=BF_NP)
    for hp in range(HP):
        sel[2 * hp, hp * 128 + 64:hp * 128 + 128] = 1
        sel[2 * hp + 1, hp * 128:hp * 128 + 64] = 1
    shared = {
        "wq_t": _tile_weight_q(wq),
        "wk_t": _tile_weight_kv(wk),
        "wv_t": _tile_weight_kv(wv),
        "wp_t": _tile_weight_q(w_proj),
        "sel_t": sel,
        "ident_t": np.eye(128, dtype=BF_NP),
    }
    return [_core_inputs(x, shared, *divmod(core, 4)) for core in range(NCORES)]


def _get_nc():
    if "nc" not in _CACHED:
        _CACHED["nc"] = _build_bass()
    return _CACHED["nc"]


def _run(x, wq, wk, wv, w_proj, trace=False, **kw):
    nc = _get_nc()
    in_maps = _prep_all(np.asarray(x, np.float32), np.asarray(wq, np.float32),
                        np.asarray(wk, np.float32), np.asarray(wv, np.float32),
                        np.asarray(w_proj, np.float32))
    res = run_bass_kernel_spmd(nc, in_maps, list(range(NCORES)), trace=trace,
                               **kw)
    out = np.empty((B, T, D_MODEL), np.float32)
    for core in range(NCORES):
        b, c = divmod(core, 4)
        out[b, c * TQ:(c + 1) * TQ] = np.asarray(res.results[core]["out"],
                                                 np.float32)
    return out, res


def kernel(x, wq, wk, wv, w_proj):
    out, _ = _run(x, wq, wk, wv, w_proj)
    return out

